# revision 14
# baseline (speedup 1.0000x reference)
"""Trainium2 Bass kernel for CrossCAM: cross channel-attention + 1x1 conv.

Reference computation (per batch b, C=64, N=H*W=16384):
    E_t = t_v @ t_v.T                     [C, C]   (t_v = template[b] as [C, N])
    E_r = r_v @ r_v.T
    attn_x = softmax(rowmax(E_x) - E_x)   rows; == exp(rowmin-E)/sum(exp(rowmin-E))
    t_out = gamma * (r_attn @ t_v) + t_v
    r_out = omega * (t_attn @ r_v) + r_v
    out   = conv_w @ concat(t_out, r_out) + conv_b        [64, N]

Key algebraic restructuring: the 1x1 conv distributes over the residual, so
    out = M_t @ t_v + M_r @ r_v + conv_b
    M_t = gamma * (w1 @ r_attn) + w1,   M_r = omega * (w2 @ t_attn) + w2
with w1 = conv_w[:, :64], w2 = conv_w[:, 64:].  Only ONE streaming pass over
the big tensors is needed; everything attention-related is 64x64.

Data layout on device ("split" layout): each [64, 16384] map is held in SBUF
as [128, 8192]: partition p = h*64+c holds t_v[c, h*8192:(h+1)*8192].  The
final matmul then runs with full K=128 using block-diagonal weights
W_x = blockdiag(M_xT, M_xT) [128, 128], and out128 in the same split layout
is contiguous-compatible with the HBM output tensor.

Sharding: pure data parallel, 2 batches per core on 8 cores.

When gamma == omega == 0 (the spec's input fill), M_t = w1 and M_r = w2 are
input constants: the attention pipeline is mathematically irrelevant (it is
multiplied by zero), so a fast program that skips it is exact.  The general
program computes the full attention path on device.
"""

import numpy as np

import concourse.tile as tile
from concourse import bacc, mybir
from concourse import bass_utils

F32 = mybir.dt.float32

B, C, H, W = 16, 64, 128, 128
N = H * W          # 16384
NCORES = 8
BPC = B // NCORES  # batches per core
HALF = N // 2      # 8192
CK = 512           # matmul free-dim chunk
NCHUNK = HALF // CK  # 16

_programs: dict[tuple, object] = {}

# DMA engine knobs (A/B-tested on hardware):
#   "sync"/"scalar" = HWDGE rings, "gpsimd" = SWDGE
LOAD_ENGINE = "sync"
STORE_ENGINE = "scalar"
# I/O + PE dtype for the fast (gamma==omega==0) path: "f16" halves HBM
# traffic on every stream (in and out) and runs the PE at 1 cyc/row;
# fp16 rounding is ~5e-4 rel err, far inside the 2e-2 gate.  "f32" /
# "f32r" keep full-precision I/O (f32r relaxes only the PE).
MM_DTYPE = "f16"
# Store chunk width in CK units (1 = per-bank stores, 2 = [128, 1024])
OC_WIDE = 2
# Fast path: load pieces per (map, phase); 1 = 1 MiB fp16 pieces
LQ = 1


def _qw():
    return HALF // LQ


def _build_fast_program():
    """gamma == omega == 0 path: out = conv_w @ concat(t, r) + bias.

    Stacked layout: X[128, 32768] holds t channels on partitions 0:64 and r
    channels on 64:128, free axis = (batch, m).  The whole conv is then ONE
    stationary weight Wc = conv_w.T [128, 64] for every matmul.  Each PSUM
    bank takes two matmuls at output partition offsets 0/64 (m and m+4096),
    so copies and stores run full 128 partitions; the store layout
    (j o) (g n) is contiguous per partition in HBM.  fp16 end to end
    (fp32 PSUM accumulate): halves HBM traffic, PE at 1 cyc/row.
    """
    nc = bacc.Bacc(
        "TRN2",
        target_bir_lowering=False,
        debug=False,
        enable_asserts=False,
        num_devices=NCORES,
    )
    DT = mybir.dt.float16 if MM_DTYPE == "f16" else F32
    t_in = nc.dram_tensor("t_in", [BPC, C, N], DT, kind="ExternalInput").ap()
    r_in = nc.dram_tensor("r_in", [BPC, C, N], DT, kind="ExternalInput").ap()
    wc_d = nc.dram_tensor("wc", [128, C], DT, kind="ExternalInput").ap()
    bias_d = nc.dram_tensor("bias2", [128, 1], F32, kind="ExternalInput").ap()
    out = nc.dram_tensor("out", [BPC, C, N], DT, kind="ExternalOutput").ap()

    Ident = mybir.ActivationFunctionType.Identity
    NPH = 2 * BPC        # phases = (batch, m-half)
    PH = N // 2          # 8192 free columns per phase
    HP = PH // 2         # 4096: psum partition-half pairing offset
    NU = HP // CK        # 8 psum units per phase

    with tile.TileContext(nc) as tc:
        from contextlib import ExitStack

        with ExitStack() as ctx:
            const = ctx.enter_context(tc.tile_pool(name="const", bufs=1))
            xp = ctx.enter_context(tc.tile_pool(name="x", bufs=1))
            pspool = ctx.enter_context(tc.tile_pool(name="ps", bufs=8, space="PSUM"))
            ocpool = ctx.enter_context(tc.tile_pool(name="oc", bufs=2))

            Wc = const.tile([128, C], DT, tag="Wc")
            nc.gpsimd.dma_start(Wc[:], wc_d[:])
            bias_sb = const.tile([128, 1], F32, tag="bias")
            nc.gpsimd.dma_start(bias_sb[:], bias_d[:])

            X = xp.tile([128, BPC * N], DT, tag="X")
            ld = getattr(nc, LOAD_ENGINE)
            LW = PH // LQ
            for p in range(NPH):
                b, g2 = divmod(p, 2)
                for q in range(LQ):
                    o0 = g2 * PH + q * LW
                    sl = slice(p * PH + q * LW, p * PH + (q + 1) * LW)
                    # t -> partitions 0:64 (even SDMA engines), r -> 64:128
                    # (odd engines): adjacent pairs drive all 16 engines.
                    ld.dma_start(X[0:64, sl], t_in[b, :, o0 : o0 + LW])
                    ld.dma_start(X[64:128, sl], r_in[b, :, o0 : o0 + LW])

            st = getattr(nc, STORE_ENGINE)
            for p in range(NPH):
                b, g2 = divmod(p, 2)
                base = p * PH
                oc = ocpool.tile([128, HP], DT, tag="oc")
                for j in range(NU):
                    ps = pspool.tile([128, CK], F32, tag="ps")
                    nc.tensor.matmul(
                        ps[0:64, :], Wc[:],
                        X[:, base + CK * j : base + CK * (j + 1)],
                        start=True, stop=True,
                    )
                    nc.tensor.matmul(
                        ps[64:128, :], Wc[:],
                        X[:, base + HP + CK * j : base + HP + CK * (j + 1)],
                        start=True, stop=True,
                    )
                    osl = oc[:, CK * j : CK * (j + 1)]
                    if j % 2 == 0:
                        nc.scalar.activation(
                            osl, ps[:], Ident, bias=bias_sb[:], scale=1.0
                        )
                    else:
                        nc.vector.tensor_scalar_add(osl, ps[:], bias_sb[:])
                m0 = g2 * PH
                # partitions 0:64 / 64:128 -> complementary SDMA engine
                # halves: the two stores drain concurrently.
                st.dma_start(out[b, :, m0 : m0 + HP], oc[0:64, :])
                st.dma_start(out[b, :, m0 + HP : m0 + 2 * HP], oc[64:128, :])

    nc.compile()
    return nc


def _build_program(with_attn: bool):
    if not with_attn:
        return _build_fast_program()
    nc = bacc.Bacc(
        "TRN2",
        target_bir_lowering=False,
        debug=False,
        enable_asserts=False,
        num_devices=NCORES,
    )
    # float32r = same 4-byte fp32 bits, but the PE runs 1 cycle/row (vs 4
    # for strict fp32) at free-dim >= 256, with relaxed internal rounding.
    # float16 additionally halves the HBM bytes of every stream.
    # The whole produce-consume chain must carry the dtype.
    if with_attn:
        MMDT = F32
    elif MM_DTYPE == "f16":
        MMDT = mybir.dt.float16
    elif MM_DTYPE == "f32r":
        MMDT = mybir.dt.float32r
    else:
        MMDT = F32
    ODT = mybir.dt.float16 if (MM_DTYPE == "f16" and not with_attn) else F32
    t_in = nc.dram_tensor("t_in", [BPC, C, N], MMDT, kind="ExternalInput").ap()
    r_in = nc.dram_tensor("r_in", [BPC, C, N], MMDT, kind="ExternalInput").ap()
    wt0 = nc.dram_tensor("wt0", [128, 128], MMDT, kind="ExternalInput").ap()
    wr0 = nc.dram_tensor("wr0", [128, 128], MMDT, kind="ExternalInput").ap()
    bias2 = nc.dram_tensor("bias2", [128, 1], F32, kind="ExternalInput").ap()
    if with_attn:
        cwt1_d = nc.dram_tensor("cwt1", [C, C], F32, kind="ExternalInput").ap()
        cwt2_d = nc.dram_tensor("cwt2", [C, C], F32, kind="ExternalInput").ap()
        gam_d = nc.dram_tensor("gam2", [128, 1], F32, kind="ExternalInput").ap()
        omg_d = nc.dram_tensor("omg2", [128, 1], F32, kind="ExternalInput").ap()
        ident_d = nc.dram_tensor("ident", [128, 128], F32, kind="ExternalInput").ap()
    out = nc.dram_tensor("out", [BPC, C, N], ODT, kind="ExternalOutput").ap()

    Exp = mybir.ActivationFunctionType.Exp
    Ident = mybir.ActivationFunctionType.Identity

    with tile.TileContext(nc) as tc:
        from contextlib import ExitStack

        with ExitStack() as ctx:
            const = ctx.enter_context(tc.tile_pool(name="const", bufs=1))
            vpool = ctx.enter_context(tc.tile_pool(name="v", bufs=2))
            pspool = ctx.enter_context(
                tc.tile_pool(name="ps", bufs=8 if not with_attn else 4, space="PSUM")
            )
            ocpool = ctx.enter_context(tc.tile_pool(name="oc", bufs=4))
            if with_attn:
                tppool = ctx.enter_context(tc.tile_pool(name="tp", bufs=2, space="PSUM"))
                egpool = ctx.enter_context(tc.tile_pool(name="eg", bufs=1, space="PSUM"))
                p1pool = ctx.enter_context(tc.tile_pool(name="p1", bufs=1, space="PSUM"))
                atpool = ctx.enter_context(tc.tile_pool(name="at", bufs=3))
                smpool = ctx.enter_context(tc.tile_pool(name="sm", bufs=2))

            cld = nc.gpsimd if not with_attn else nc.sync
            Wt = const.tile([128, 128], MMDT, tag="Wt")
            cld.dma_start(Wt[:], wt0[:])
            Wr = const.tile([128, 128], MMDT, tag="Wr")
            cld.dma_start(Wr[:], wr0[:])
            bias_sb = const.tile([128, 1], F32, tag="bias")
            cld.dma_start(bias_sb[:], bias2[:])
            if with_attn:
                cwt1 = const.tile([C, C], F32, tag="cwt1")
                nc.sync.dma_start(cwt1[:], cwt1_d[:])
                cwt2 = const.tile([C, C], F32, tag="cwt2")
                nc.sync.dma_start(cwt2[:], cwt2_d[:])
                gam = const.tile([128, 1], F32, tag="gam")
                nc.sync.dma_start(gam[:], gam_d[:])
                omg = const.tile([128, 1], F32, tag="omg")
                nc.sync.dma_start(omg[:], omg_d[:])
                ident = const.tile([128, 128], F32, tag="ident")
                nc.sync.dma_start(ident[:], ident_d[:])

            for i in range(BPC):
                ld = getattr(nc, LOAD_ENGINE if LOAD_ENGINE != "alt" else "sync")
                if with_attn:
                    # block-split layout: partition h*64+c <- v[c, h*HALF+n]
                    t128 = vpool.tile([128, HALF], MMDT, tag="t")
                    r128 = vpool.tile([128, HALF], MMDT, tag="r")
                    ld.dma_start(t128[0:64, :], t_in[i, :, 0:HALF])
                    ld.dma_start(t128[64:128, :], t_in[i, :, HALF:N])
                    ld.dma_start(r128[0:64, :], r_in[i, :, 0:HALF])
                    ld.dma_start(r128[64:128, :], r_in[i, :, HALF:N])
                else:
                    # interleaved layout: partition 2c+h <- v[c, h*HALF+n].
                    # One DMA covers all 128 partitions -> all 16 SBUF AXI
                    # ports engage concurrently (the split form above only
                    # drives half the ports per transfer).  Each map is
                    # loaded as LQ quarter tiles so the first matmuls can
                    # start as soon as the first quarter lands.
                    QW = _qw()
                    t_il = t_in[i].rearrange("c (h n) -> (c h) n", h=2)
                    r_il = r_in[i].rearrange("c (h n) -> (c h) n", h=2)
                    tq, rq = [], []
                    for q in range(LQ):
                        if LOAD_ENGINE == "alt":
                            ld = nc.sync if q % 2 == 0 else nc.scalar
                        tt = vpool.tile([128, QW], MMDT, tag=f"t{q}")
                        ld.dma_start(tt[:], t_il[:, QW * q : QW * (q + 1)])
                        tq.append(tt)
                        rr = vpool.tile([128, QW], MMDT, tag=f"r{q}")
                        ld.dma_start(rr[:], r_il[:, QW * q : QW * (q + 1)])
                        rq.append(rr)

                if with_attn:
                    attn = {}
                    for name, v128 in (("t", t128), ("r", r128)):
                        # E_grand[a, b] = sum_f v128[a, f] v128[b, f], via
                        # PE-transposed chunks; E = diag-fold of E_grand.
                        eg_ps = egpool.tile([128, 128], F32, tag="eg")
                        for g in range(HALF // CK):
                            tp = tppool.tile([128, CK], F32, tag="tp")
                            for q in range(4):
                                k = 4 * g + q
                                nc.tensor.transpose(
                                    tp[:, 128 * q : 128 * (q + 1)],
                                    v128[:, 128 * k : 128 * (k + 1)],
                                    ident[:],
                                )
                            at = atpool.tile([128, CK], F32, tag="at")
                            nc.scalar.copy(at[:], tp[:])
                            for q in range(4):
                                k = 4 * g + q
                                sl = at[:, 128 * q : 128 * (q + 1)]
                                nc.tensor.matmul(
                                    eg_ps[:],
                                    sl,
                                    sl,
                                    start=(k == 0),
                                    stop=(k == HALF // 128 - 1),
                                )
                        egs = smpool.tile([128, 128], F32, tag="egs")
                        nc.vector.tensor_copy(egs[:], eg_ps[:])
                        eglow = smpool.tile([C, C], F32, tag="eglow")
                        nc.sync.dma_start(eglow[:], egs[64:128, 64:128])
                        e = smpool.tile([C, C], F32, tag="e")
                        nc.vector.tensor_add(e[:], egs[0:64, 0:64], eglow[:])
                        # softmax(rowmax(E)-E) == exp(rowmin(E)-E)/sum(...)
                        rmin = smpool.tile([C, 1], F32, tag="rmin")
                        nc.vector.tensor_reduce(
                            rmin[:], e[:], axis=mybir.AxisListType.X,
                            op=mybir.AluOpType.min,
                        )
                        p = smpool.tile([C, C], F32, tag="p")
                        rsum = smpool.tile([C, 1], F32, tag="rsum")
                        nc.scalar.activation(
                            p[:], e[:], Exp, bias=rmin[:], scale=-1.0,
                            accum_out=rsum[:],
                        )
                        rinv = smpool.tile([C, 1], F32, tag="rinv")
                        nc.vector.reciprocal(rinv[:], rsum[:])
                        a = smpool.tile([C, C], F32, tag=f"attn_{name}")
                        nc.vector.tensor_scalar_mul(a[:], p[:], rinv[:])
                        attn[name] = a

                    # W_x diag blocks: M_tT = gamma*(w1@r_attn).T + w1T, etc.
                    # (w1@r_attn).T = r_attn.T.T @ w1T = matmul(lhsT=r_attn, rhs=w1T)
                    for wtile, a, cw, g_ap in (
                        (Wt, attn["r"], cwt1, gam),
                        (Wr, attn["t"], cwt2, omg),
                    ):
                        p1 = p1pool.tile([C, C], F32, tag="p1")
                        nc.tensor.matmul(p1[:], a[:], cw[:], start=True, stop=True)
                        tmp = smpool.tile([C, C], F32, tag="tmp")
                        nc.vector.tensor_scalar_mul(tmp[:], p1[:], g_ap[0:64, :])
                        nc.vector.tensor_add(wtile[0:64, 0:64], tmp[:], cw[:])
                        nc.sync.dma_start(wtile[64:128, 64:128], wtile[0:64, 0:64])

                # out128 = Wt.T @ t128 + Wr.T @ r128 + bias (same layout as v)
                st = getattr(nc, STORE_ENGINE)
                out_il = None
                if not with_attn:
                    out_il = out[i].rearrange("c (h n) -> (c h) n", h=2)

                def t_chunk(j):
                    if with_attn:
                        return t128[:, CK * j : CK * (j + 1)]
                    o = CK * j
                    qw = _qw()
                    return tq[o // qw][:, o % qw : o % qw + CK]

                def r_chunk(j):
                    if with_attn:
                        return r128[:, CK * j : CK * (j + 1)]
                    o = CK * j
                    qw = _qw()
                    return rq[o // qw][:, o % qw : o % qw + CK]

                group = max(_qw() // CK, OC_WIDE) if not with_attn else 4
                for g in range(NCHUNK // group):
                    pss = []
                    for q in range(group):
                        j = group * g + q
                        ps = pspool.tile([128, CK], F32, tag="ps")
                        nc.tensor.matmul(
                            ps[:], Wt[:], t_chunk(j),
                            start=True, stop=False,
                        )
                        pss.append((j, ps))
                    for j, ps in pss:
                        nc.tensor.matmul(
                            ps[:], Wr[:], r_chunk(j),
                            start=False, stop=True,
                        )
                    oc = None
                    for idx, (j, ps) in enumerate(pss):
                        w = idx % OC_WIDE
                        if w == 0:
                            oc = ocpool.tile([128, CK * OC_WIDE], ODT, tag="oc")
                        nc.scalar.activation(
                            oc[:, CK * w : CK * (w + 1)], ps[:],
                            Ident, bias=bias_sb[:], scale=1.0,
                        )
                        if w < OC_WIDE - 1:
                            continue
                        j0 = j - (OC_WIDE - 1)
                        span = CK * OC_WIDE
                        if with_attn:
                            st.dma_start(
                                out[i, :, CK * j0 : CK * j0 + span],
                                oc[0:64, :],
                            )
                            st.dma_start(
                                out[i, :, HALF + CK * j0 : HALF + CK * j0 + span],
                                oc[64:128, :],
                            )
                        else:
                            st.dma_start(
                                out_il[:, CK * j0 : CK * j0 + span], oc[:]
                            )

    nc.compile()
    return nc


def _get_program(with_attn: bool):
    key = (with_attn, LOAD_ENGINE, STORE_ENGINE, MM_DTYPE, OC_WIDE)
    prog = _programs.get(key)
    if prog is None:
        prog = _build_program(with_attn)
        _programs[key] = prog
    return prog


def make_in_maps(template_map, roi_map, gamma, omega, conv_w, conv_b):
    """Host-side prep: per-core input dicts + which program variant to use."""
    template_map = np.ascontiguousarray(np.asarray(template_map, dtype=np.float32))
    roi_map = np.ascontiguousarray(np.asarray(roi_map, dtype=np.float32))
    conv_w = np.asarray(conv_w, dtype=np.float32)
    conv_b = np.asarray(conv_b, dtype=np.float32)
    g = float(np.asarray(gamma).reshape(-1)[0])
    o = float(np.asarray(omega).reshape(-1)[0])
    with_attn = not (g == 0.0 and o == 0.0)

    w1T = np.ascontiguousarray(conv_w[:, :C].T)  # [c, o]
    w2T = np.ascontiguousarray(conv_w[:, C:].T)
    if with_attn:
        # block-split layout: W[h*64+c, h*64+o] = wT[c, o]
        wt0 = np.zeros((128, 128), np.float32)
        wt0[:64, :64] = w1T
        wt0[64:, 64:] = w1T
        wr0 = np.zeros((128, 128), np.float32)
        wr0[:64, :64] = w2T
        wr0[64:, 64:] = w2T
        bias2 = np.ascontiguousarray(np.tile(conv_b, 2)[:, None])  # [128, 1]
    io_np = np.float32
    if with_attn:
        common = {
            "wt0": wt0,
            "wr0": wr0,
            "bias2": np.ascontiguousarray(np.tile(conv_b, 2)[:, None]),
            "cwt1": w1T,
            "cwt2": w2T,
            "gam2": np.full((128, 1), g, np.float32),
            "omg2": np.full((128, 1), o, np.float32),
            "ident": np.eye(128, dtype=np.float32),
        }
    else:
        # stacked layout: Wc = conv_w.T [128, 64]; bias per (j, o) partition
        if MM_DTYPE == "f16":
            io_np = np.float16
        common = {
            "wc": np.ascontiguousarray(conv_w.T).astype(io_np),
            "bias2": np.ascontiguousarray(np.tile(conv_b, 2)[:, None]),
        }

    tm = template_map.reshape(B, C, N).astype(io_np, copy=False)
    rm = roi_map.reshape(B, C, N).astype(io_np, copy=False)
    in_maps = [
        dict(
            common,
            t_in=tm[BPC * i : BPC * (i + 1)],
            r_in=rm[BPC * i : BPC * (i + 1)],
        )
        for i in range(NCORES)
    ]
    return in_maps, with_attn


def kernel(template_map, roi_map, gamma, omega, conv_w, conv_b):
    in_maps, with_attn = make_in_maps(
        template_map, roi_map, gamma, omega, conv_w, conv_b
    )
    nc = _get_program(with_attn)
    res = bass_utils.run_bass_kernel_spmd(nc, in_maps, core_ids=list(range(NCORES)))
    outp = np.concatenate(
        [np.asarray(res.results[i]["out"], dtype=np.float32) for i in range(NCORES)],
        axis=0,
    )
    return outp.reshape(B, C, H, W)



# revision 19
# speedup vs baseline: 1.3888x; 1.3888x over previous
"""Trainium2 Bass kernel for CrossCAM: cross channel-attention + 1x1 conv.

Reference computation (per batch b, C=64, N=H*W=16384):
    E_t = t_v @ t_v.T                     [C, C]   (t_v = template[b] as [C, N])
    E_r = r_v @ r_v.T
    attn_x = softmax(rowmax(E_x) - E_x)   rows; == exp(rowmin-E)/sum(exp(rowmin-E))
    t_out = gamma * (r_attn @ t_v) + t_v
    r_out = omega * (t_attn @ r_v) + r_v
    out   = conv_w @ concat(t_out, r_out) + conv_b        [64, N]

Key algebraic restructuring: the 1x1 conv distributes over the residual, so
    out = M_t @ t_v + M_r @ r_v + conv_b
    M_t = gamma * (w1 @ r_attn) + w1,   M_r = omega * (w2 @ t_attn) + w2
with w1 = conv_w[:, :64], w2 = conv_w[:, 64:].  Only ONE streaming pass over
the big tensors is needed; everything attention-related is 64x64.

Data layout on device ("split" layout): each [64, 16384] map is held in SBUF
as [128, 8192]: partition p = h*64+c holds t_v[c, h*8192:(h+1)*8192].  The
final matmul then runs with full K=128 using block-diagonal weights
W_x = blockdiag(M_xT, M_xT) [128, 128], and out128 in the same split layout
is contiguous-compatible with the HBM output tensor.

Sharding: pure data parallel, 2 batches per core on 8 cores.

When gamma == omega == 0 (the spec's input fill), M_t = w1 and M_r = w2 are
input constants: the attention pipeline is mathematically irrelevant (it is
multiplied by zero), so a fast program that skips it is exact.  The general
program computes the full attention path on device.
"""

import numpy as np

import concourse.tile as tile
from concourse import bacc, mybir
from concourse import bass_utils

F32 = mybir.dt.float32

B, C, H, W = 16, 64, 128, 128
N = H * W          # 16384
NCORES = 8
BPC = B // NCORES  # batches per core
HALF = N // 2      # 8192
CK = 512           # matmul free-dim chunk
NCHUNK = HALF // CK  # 16

_programs: dict[tuple, object] = {}

# DMA engine knobs (A/B-tested on hardware):
#   "sync"/"scalar" = HWDGE rings, "gpsimd" = SWDGE
LOAD_ENGINE = "sync"
STORE_ENGINE = "scalar"
# I/O + PE dtype for the fast (gamma==omega==0) path: "f16" halves HBM
# traffic on every stream (in and out) and runs the PE at 1 cyc/row;
# fp16 rounding is ~5e-4 rel err, far inside the 2e-2 gate.  "f32" /
# "f32r" keep full-precision I/O (f32r relaxes only the PE).
MM_DTYPE = "f16"
# Store chunk width in CK units (1 = per-bank stores, 2 = [128, 1024])
OC_WIDE = 2
# Fast path: load pieces per (map, phase); 1 = 1 MiB fp16 pieces
LQ = 1


def _qw():
    return HALF // LQ


def _build_fast_program():
    """gamma == omega == 0 path: out = conv_w @ concat(t, r) + bias.

    Stacked layout: X[128, 32768] holds t channels on partitions 0:64 and r
    channels on 64:128, free axis = (batch, m).  The whole conv is then ONE
    stationary weight Wc = conv_w.T [128, 64] for every matmul.  Each PSUM
    bank takes two matmuls at output partition offsets 0/64 (m and m+4096),
    so copies and stores run full 128 partitions; the store layout
    (j o) (g n) is contiguous per partition in HBM.  fp16 end to end
    (fp32 PSUM accumulate): halves HBM traffic, PE at 1 cyc/row.
    """
    nc = bacc.Bacc(
        "TRN2",
        target_bir_lowering=False,
        debug=False,
        enable_asserts=False,
        num_devices=NCORES,
    )
    DT = mybir.dt.float16 if MM_DTYPE == "f16" else F32
    # x_in packs t (rows 0:64) and r (rows 64:128) host-side so every load
    # is a full 128-partition transfer (keeps SDMA engine<->port affinity;
    # 64-partition transfers measured ~55% of peak).  out_s is a scratch
    # layout [b, g2, j2, o, n] matching the PSUM partition order (j2 o);
    # the host unshard permutes it back to [b, o, m].
    x_in = nc.dram_tensor("x_in", [BPC, 128, N], DT, kind="ExternalInput").ap()
    wc_d = nc.dram_tensor("wc", [128, C], DT, kind="ExternalInput").ap()
    bias_d = nc.dram_tensor("bias2", [128, 1], F32, kind="ExternalInput").ap()
    out = nc.dram_tensor(
        "out", [BPC, 2, 2, C, N // 4], DT, kind="ExternalOutput"
    ).ap()

    Ident = mybir.ActivationFunctionType.Identity
    NPH = 2 * BPC        # phases = (batch, m-half)
    PH = N // 2          # 8192 free columns per phase
    HP = PH // 2         # 4096: psum partition-half pairing offset
    NU = HP // CK        # 8 psum units per phase

    with tile.TileContext(nc) as tc:
        from contextlib import ExitStack

        with ExitStack() as ctx:
            const = ctx.enter_context(tc.tile_pool(name="const", bufs=1))
            xp = ctx.enter_context(tc.tile_pool(name="x", bufs=1))
            pspool = ctx.enter_context(tc.tile_pool(name="ps", bufs=8, space="PSUM"))
            ocpool = ctx.enter_context(tc.tile_pool(name="oc", bufs=2))

            Wc = const.tile([128, C], DT, tag="Wc")
            nc.gpsimd.dma_start(Wc[:], wc_d[:])
            bias_sb = const.tile([128, 1], F32, tag="bias")
            nc.gpsimd.dma_start(bias_sb[:], bias_d[:])

            X = xp.tile([128, BPC * N], DT, tag="X")
            ld = getattr(nc, LOAD_ENGINE)
            LW = PH // LQ
            for p in range(NPH):
                b, g2 = divmod(p, 2)
                for q in range(LQ):
                    o0 = g2 * PH + q * LW
                    sl = slice(p * PH + q * LW, p * PH + (q + 1) * LW)
                    ld.dma_start(X[:, sl], x_in[b, :, o0 : o0 + LW])

            st = getattr(nc, STORE_ENGINE)
            for p in range(NPH):
                b, g2 = divmod(p, 2)
                base = p * PH
                oc = ocpool.tile([128, HP], DT, tag="oc")
                for j in range(NU):
                    ps = pspool.tile([128, CK], F32, tag="ps")
                    nc.tensor.matmul(
                        ps[0:64, :], Wc[:],
                        X[:, base + CK * j : base + CK * (j + 1)],
                        start=True, stop=True,
                    )
                    nc.tensor.matmul(
                        ps[64:128, :], Wc[:],
                        X[:, base + HP + CK * j : base + HP + CK * (j + 1)],
                        start=True, stop=True,
                    )
                    osl = oc[:, CK * j : CK * (j + 1)]
                    if j % 2 == 0:
                        nc.scalar.activation(
                            osl, ps[:], Ident, bias=bias_sb[:], scale=1.0
                        )
                    else:
                        nc.vector.tensor_scalar_add(osl, ps[:], bias_sb[:])
                st.dma_start(
                    out[b, g2].rearrange("j o n -> (j o) n"), oc[:]
                )

    nc.compile()
    return nc


def _build_program(with_attn: bool):
    if not with_attn:
        return _build_fast_program()
    nc = bacc.Bacc(
        "TRN2",
        target_bir_lowering=False,
        debug=False,
        enable_asserts=False,
        num_devices=NCORES,
    )
    # float32r = same 4-byte fp32 bits, but the PE runs 1 cycle/row (vs 4
    # for strict fp32) at free-dim >= 256, with relaxed internal rounding.
    # float16 additionally halves the HBM bytes of every stream.
    # The whole produce-consume chain must carry the dtype.
    if with_attn:
        MMDT = F32
    elif MM_DTYPE == "f16":
        MMDT = mybir.dt.float16
    elif MM_DTYPE == "f32r":
        MMDT = mybir.dt.float32r
    else:
        MMDT = F32
    ODT = mybir.dt.float16 if (MM_DTYPE == "f16" and not with_attn) else F32
    t_in = nc.dram_tensor("t_in", [BPC, C, N], MMDT, kind="ExternalInput").ap()
    r_in = nc.dram_tensor("r_in", [BPC, C, N], MMDT, kind="ExternalInput").ap()
    wt0 = nc.dram_tensor("wt0", [128, 128], MMDT, kind="ExternalInput").ap()
    wr0 = nc.dram_tensor("wr0", [128, 128], MMDT, kind="ExternalInput").ap()
    bias2 = nc.dram_tensor("bias2", [128, 1], F32, kind="ExternalInput").ap()
    if with_attn:
        cwt1_d = nc.dram_tensor("cwt1", [C, C], F32, kind="ExternalInput").ap()
        cwt2_d = nc.dram_tensor("cwt2", [C, C], F32, kind="ExternalInput").ap()
        gam_d = nc.dram_tensor("gam2", [128, 1], F32, kind="ExternalInput").ap()
        omg_d = nc.dram_tensor("omg2", [128, 1], F32, kind="ExternalInput").ap()
        ident_d = nc.dram_tensor("ident", [128, 128], F32, kind="ExternalInput").ap()
    out = nc.dram_tensor("out", [BPC, C, N], ODT, kind="ExternalOutput").ap()

    Exp = mybir.ActivationFunctionType.Exp
    Ident = mybir.ActivationFunctionType.Identity

    with tile.TileContext(nc) as tc:
        from contextlib import ExitStack

        with ExitStack() as ctx:
            const = ctx.enter_context(tc.tile_pool(name="const", bufs=1))
            vpool = ctx.enter_context(tc.tile_pool(name="v", bufs=2))
            pspool = ctx.enter_context(
                tc.tile_pool(name="ps", bufs=8 if not with_attn else 4, space="PSUM")
            )
            ocpool = ctx.enter_context(tc.tile_pool(name="oc", bufs=4))
            if with_attn:
                tppool = ctx.enter_context(tc.tile_pool(name="tp", bufs=2, space="PSUM"))
                egpool = ctx.enter_context(tc.tile_pool(name="eg", bufs=1, space="PSUM"))
                p1pool = ctx.enter_context(tc.tile_pool(name="p1", bufs=1, space="PSUM"))
                atpool = ctx.enter_context(tc.tile_pool(name="at", bufs=3))
                smpool = ctx.enter_context(tc.tile_pool(name="sm", bufs=2))

            cld = nc.gpsimd if not with_attn else nc.sync
            Wt = const.tile([128, 128], MMDT, tag="Wt")
            cld.dma_start(Wt[:], wt0[:])
            Wr = const.tile([128, 128], MMDT, tag="Wr")
            cld.dma_start(Wr[:], wr0[:])
            bias_sb = const.tile([128, 1], F32, tag="bias")
            cld.dma_start(bias_sb[:], bias2[:])
            if with_attn:
                cwt1 = const.tile([C, C], F32, tag="cwt1")
                nc.sync.dma_start(cwt1[:], cwt1_d[:])
                cwt2 = const.tile([C, C], F32, tag="cwt2")
                nc.sync.dma_start(cwt2[:], cwt2_d[:])
                gam = const.tile([128, 1], F32, tag="gam")
                nc.sync.dma_start(gam[:], gam_d[:])
                omg = const.tile([128, 1], F32, tag="omg")
                nc.sync.dma_start(omg[:], omg_d[:])
                ident = const.tile([128, 128], F32, tag="ident")
                nc.sync.dma_start(ident[:], ident_d[:])

            for i in range(BPC):
                ld = getattr(nc, LOAD_ENGINE if LOAD_ENGINE != "alt" else "sync")
                if with_attn:
                    # block-split layout: partition h*64+c <- v[c, h*HALF+n]
                    t128 = vpool.tile([128, HALF], MMDT, tag="t")
                    r128 = vpool.tile([128, HALF], MMDT, tag="r")
                    ld.dma_start(t128[0:64, :], t_in[i, :, 0:HALF])
                    ld.dma_start(t128[64:128, :], t_in[i, :, HALF:N])
                    ld.dma_start(r128[0:64, :], r_in[i, :, 0:HALF])
                    ld.dma_start(r128[64:128, :], r_in[i, :, HALF:N])
                else:
                    # interleaved layout: partition 2c+h <- v[c, h*HALF+n].
                    # One DMA covers all 128 partitions -> all 16 SBUF AXI
                    # ports engage concurrently (the split form above only
                    # drives half the ports per transfer).  Each map is
                    # loaded as LQ quarter tiles so the first matmuls can
                    # start as soon as the first quarter lands.
                    QW = _qw()
                    t_il = t_in[i].rearrange("c (h n) -> (c h) n", h=2)
                    r_il = r_in[i].rearrange("c (h n) -> (c h) n", h=2)
                    tq, rq = [], []
                    for q in range(LQ):
                        if LOAD_ENGINE == "alt":
                            ld = nc.sync if q % 2 == 0 else nc.scalar
                        tt = vpool.tile([128, QW], MMDT, tag=f"t{q}")
                        ld.dma_start(tt[:], t_il[:, QW * q : QW * (q + 1)])
                        tq.append(tt)
                        rr = vpool.tile([128, QW], MMDT, tag=f"r{q}")
                        ld.dma_start(rr[:], r_il[:, QW * q : QW * (q + 1)])
                        rq.append(rr)

                if with_attn:
                    attn = {}
                    for name, v128 in (("t", t128), ("r", r128)):
                        # E_grand[a, b] = sum_f v128[a, f] v128[b, f], via
                        # PE-transposed chunks; E = diag-fold of E_grand.
                        eg_ps = egpool.tile([128, 128], F32, tag="eg")
                        for g in range(HALF // CK):
                            tp = tppool.tile([128, CK], F32, tag="tp")
                            for q in range(4):
                                k = 4 * g + q
                                nc.tensor.transpose(
                                    tp[:, 128 * q : 128 * (q + 1)],
                                    v128[:, 128 * k : 128 * (k + 1)],
                                    ident[:],
                                )
                            at = atpool.tile([128, CK], F32, tag="at")
                            nc.scalar.copy(at[:], tp[:])
                            for q in range(4):
                                k = 4 * g + q
                                sl = at[:, 128 * q : 128 * (q + 1)]
                                nc.tensor.matmul(
                                    eg_ps[:],
                                    sl,
                                    sl,
                                    start=(k == 0),
                                    stop=(k == HALF // 128 - 1),
                                )
                        egs = smpool.tile([128, 128], F32, tag="egs")
                        nc.vector.tensor_copy(egs[:], eg_ps[:])
                        eglow = smpool.tile([C, C], F32, tag="eglow")
                        nc.sync.dma_start(eglow[:], egs[64:128, 64:128])
                        e = smpool.tile([C, C], F32, tag="e")
                        nc.vector.tensor_add(e[:], egs[0:64, 0:64], eglow[:])
                        # softmax(rowmax(E)-E) == exp(rowmin(E)-E)/sum(...)
                        rmin = smpool.tile([C, 1], F32, tag="rmin")
                        nc.vector.tensor_reduce(
                            rmin[:], e[:], axis=mybir.AxisListType.X,
                            op=mybir.AluOpType.min,
                        )
                        p = smpool.tile([C, C], F32, tag="p")
                        rsum = smpool.tile([C, 1], F32, tag="rsum")
                        nc.scalar.activation(
                            p[:], e[:], Exp, bias=rmin[:], scale=-1.0,
                            accum_out=rsum[:],
                        )
                        rinv = smpool.tile([C, 1], F32, tag="rinv")
                        nc.vector.reciprocal(rinv[:], rsum[:])
                        a = smpool.tile([C, C], F32, tag=f"attn_{name}")
                        nc.vector.tensor_scalar_mul(a[:], p[:], rinv[:])
                        attn[name] = a

                    # W_x diag blocks: M_tT = gamma*(w1@r_attn).T + w1T, etc.
                    # (w1@r_attn).T = r_attn.T.T @ w1T = matmul(lhsT=r_attn, rhs=w1T)
                    for wtile, a, cw, g_ap in (
                        (Wt, attn["r"], cwt1, gam),
                        (Wr, attn["t"], cwt2, omg),
                    ):
                        p1 = p1pool.tile([C, C], F32, tag="p1")
                        nc.tensor.matmul(p1[:], a[:], cw[:], start=True, stop=True)
                        tmp = smpool.tile([C, C], F32, tag="tmp")
                        nc.vector.tensor_scalar_mul(tmp[:], p1[:], g_ap[0:64, :])
                        nc.vector.tensor_add(wtile[0:64, 0:64], tmp[:], cw[:])
                        nc.sync.dma_start(wtile[64:128, 64:128], wtile[0:64, 0:64])

                # out128 = Wt.T @ t128 + Wr.T @ r128 + bias (same layout as v)
                st = getattr(nc, STORE_ENGINE)
                out_il = None
                if not with_attn:
                    out_il = out[i].rearrange("c (h n) -> (c h) n", h=2)

                def t_chunk(j):
                    if with_attn:
                        return t128[:, CK * j : CK * (j + 1)]
                    o = CK * j
                    qw = _qw()
                    return tq[o // qw][:, o % qw : o % qw + CK]

                def r_chunk(j):
                    if with_attn:
                        return r128[:, CK * j : CK * (j + 1)]
                    o = CK * j
                    qw = _qw()
                    return rq[o // qw][:, o % qw : o % qw + CK]

                group = max(_qw() // CK, OC_WIDE) if not with_attn else 4
                for g in range(NCHUNK // group):
                    pss = []
                    for q in range(group):
                        j = group * g + q
                        ps = pspool.tile([128, CK], F32, tag="ps")
                        nc.tensor.matmul(
                            ps[:], Wt[:], t_chunk(j),
                            start=True, stop=False,
                        )
                        pss.append((j, ps))
                    for j, ps in pss:
                        nc.tensor.matmul(
                            ps[:], Wr[:], r_chunk(j),
                            start=False, stop=True,
                        )
                    oc = None
                    for idx, (j, ps) in enumerate(pss):
                        w = idx % OC_WIDE
                        if w == 0:
                            oc = ocpool.tile([128, CK * OC_WIDE], ODT, tag="oc")
                        nc.scalar.activation(
                            oc[:, CK * w : CK * (w + 1)], ps[:],
                            Ident, bias=bias_sb[:], scale=1.0,
                        )
                        if w < OC_WIDE - 1:
                            continue
                        j0 = j - (OC_WIDE - 1)
                        span = CK * OC_WIDE
                        if with_attn:
                            st.dma_start(
                                out[i, :, CK * j0 : CK * j0 + span],
                                oc[0:64, :],
                            )
                            st.dma_start(
                                out[i, :, HALF + CK * j0 : HALF + CK * j0 + span],
                                oc[64:128, :],
                            )
                        else:
                            st.dma_start(
                                out_il[:, CK * j0 : CK * j0 + span], oc[:]
                            )

    nc.compile()
    return nc


def _get_program(with_attn: bool):
    key = (with_attn, LOAD_ENGINE, STORE_ENGINE, MM_DTYPE, OC_WIDE)
    prog = _programs.get(key)
    if prog is None:
        prog = _build_program(with_attn)
        _programs[key] = prog
    return prog


def make_in_maps(template_map, roi_map, gamma, omega, conv_w, conv_b):
    """Host-side prep: per-core input dicts + which program variant to use."""
    template_map = np.ascontiguousarray(np.asarray(template_map, dtype=np.float32))
    roi_map = np.ascontiguousarray(np.asarray(roi_map, dtype=np.float32))
    conv_w = np.asarray(conv_w, dtype=np.float32)
    conv_b = np.asarray(conv_b, dtype=np.float32)
    g = float(np.asarray(gamma).reshape(-1)[0])
    o = float(np.asarray(omega).reshape(-1)[0])
    with_attn = not (g == 0.0 and o == 0.0)

    w1T = np.ascontiguousarray(conv_w[:, :C].T)  # [c, o]
    w2T = np.ascontiguousarray(conv_w[:, C:].T)
    if with_attn:
        # block-split layout: W[h*64+c, h*64+o] = wT[c, o]
        wt0 = np.zeros((128, 128), np.float32)
        wt0[:64, :64] = w1T
        wt0[64:, 64:] = w1T
        wr0 = np.zeros((128, 128), np.float32)
        wr0[:64, :64] = w2T
        wr0[64:, 64:] = w2T
        bias2 = np.ascontiguousarray(np.tile(conv_b, 2)[:, None])  # [128, 1]
    io_np = np.float32
    if with_attn:
        common = {
            "wt0": wt0,
            "wr0": wr0,
            "bias2": np.ascontiguousarray(np.tile(conv_b, 2)[:, None]),
            "cwt1": w1T,
            "cwt2": w2T,
            "gam2": np.full((128, 1), g, np.float32),
            "omg2": np.full((128, 1), o, np.float32),
            "ident": np.eye(128, dtype=np.float32),
        }
    else:
        # stacked layout: Wc = conv_w.T [128, 64]; bias per (j, o) partition
        if MM_DTYPE == "f16":
            io_np = np.float16
        common = {
            "wc": np.ascontiguousarray(conv_w.T).astype(io_np),
            "bias2": np.ascontiguousarray(np.tile(conv_b, 2)[:, None]),
        }
        x = np.empty((B, 128, N), io_np)
        x[:, :C] = template_map.reshape(B, C, N)
        x[:, C:] = roi_map.reshape(B, C, N)
        return [
            dict(common, x_in=x[BPC * i : BPC * (i + 1)]) for i in range(NCORES)
        ], with_attn

    tm = template_map.reshape(B, C, N).astype(io_np, copy=False)
    rm = roi_map.reshape(B, C, N).astype(io_np, copy=False)
    in_maps = [
        dict(
            common,
            t_in=tm[BPC * i : BPC * (i + 1)],
            r_in=rm[BPC * i : BPC * (i + 1)],
        )
        for i in range(NCORES)
    ]
    return in_maps, with_attn


def kernel(template_map, roi_map, gamma, omega, conv_w, conv_b):
    in_maps, with_attn = make_in_maps(
        template_map, roi_map, gamma, omega, conv_w, conv_b
    )
    nc = _get_program(with_attn)
    res = bass_utils.run_bass_kernel_spmd(nc, in_maps, core_ids=list(range(NCORES)))
    outs = [np.asarray(res.results[i]["out"], dtype=np.float32) for i in range(NCORES)]
    if not with_attn:
        # scratch layout [b, g2, j2, o, n] -> [b, o, m], m = (g2, j2, n)
        outs = [
            o.transpose(0, 3, 1, 2, 4).reshape(BPC, C, N) for o in outs
        ]
    outp = np.concatenate(outs, axis=0)
    return outp.reshape(B, C, H, W)

